# revision 2
# baseline (speedup 1.0000x reference)
"""GCN message-passing kernel for Trainium2, 8-core SPMD — v3.

Changes vs v2 (2,118,841 ns):
 - Sources split in 3 ranges (tiles 0-6 / 6-26 / 26-49) with 3 AllGathers
   emitted mid-MLP so the first gathers start ~90us instead of ~345us.
 - Paired descriptors: adjacent table rows (s, s+1) needed by the same
   (pair, range) group share one 2-row SWDGE descriptor (overlapping-AP
   gather, elem_step=row); measured 8.28 ns/desc regardless of 512B/1024B.
 - Chunks are shared across the 2 tiles of a pair (256-slot dstl) so the
   ceil-to-128 padding is paid once per (pair, range), not per tile.
 - Per-core balanced node->tile assignment (greedy vector bin-pack on
   per-range degrees) so the max-over-cores chunk counts track the mean.
 - Conv accumulators live in SBUF f32 (psum->sbuf add per range sweep);
   the gather stream consumes range-major (all pairs r0, then r1, ...) so
   it never stalls on a late AllGather.
"""

import os
import sys
import types

import numpy as np

P = 128


# ----------------------------------------------------------------------------
# environment shims (unchanged)
# ----------------------------------------------------------------------------

def _install_ntff_shim():
    if "antenv.axon_hooks" in sys.modules:
        return
    hook_holder = [None]
    mod = types.ModuleType("antenv.axon_hooks")
    mod.set_axon_ntff_profile_hook = lambda h: hook_holder.__setitem__(0, h)
    mod.get_axon_ntff_profile_hook = lambda: hook_holder[0]
    sys.modules["antenv.axon_hooks"] = mod
    try:
        import antenv
        antenv.axon_hooks = mod
    except ImportError:
        pass
    try:
        from trn_agent_boot.trn_boot import _ntff_profile_via_ctypes
        h = _ntff_profile_via_ctypes("/opt/axon/libaxon_pjrt.so")
        if h is not None:
            mod.set_axon_ntff_profile_hook(h)
    except Exception:
        pass


def _split_drain_waits(nc):
    import concourse.mybir as mybir
    nid = [0]
    for blk in nc.main_func.blocks:
        new_list = []
        for ins in blk.instructions:
            si = ins.sync_info
            if si is not None and ins.engine is not None:
                waits = list(si.on_wait or [])
                keep = 0 if type(ins).__name__ == "InstDrain" else 1
                if len(waits) > keep:
                    move, stay = waits[:len(waits) - keep], waits[len(waits) - keep:]
                    for w in move:
                        nid[0] += 1
                        ev = mybir.InstEventSemaphore(
                            name=f"splitwait-{nid[0]}",
                            engine=ins.engine,
                            ins=[], outs=[],
                            sync_info=mybir.SyncInfo(on_wait=[w], on_update=[]),
                        )
                        new_list.append(ev)
                    si.on_wait = stay
            new_list.append(ins)
        blk.instructions = new_list
    return nc


def _make_nc(num_devices):
    import concourse.bacc as bacc

    class PatchedBacc(bacc.Bacc):
        def compile(self):
            super().compile()
            _split_drain_waits(self)

    return PatchedBacc("TRN2", target_bir_lowering=False, debug=False,
                       num_devices=num_devices, num_swdge_queues=1)


# ----------------------------------------------------------------------------
# config
# ----------------------------------------------------------------------------

RT = [12, 20, 17]         # tiles per source range (sum = 49)
RB = [0, 12, 32, 49]      # range tile boundaries


class Cfg:
    def __init__(self, N=50000, E=800000, DF=4096, DL=256, DID=64, M=8):
        self.N, self.E, self.DF, self.DL, self.DID, self.M = N, E, DF, DL, DID, M
        self.DLX = DL + DID
        self.NS_RAW = N // M
        self.NT = -(-self.NS_RAW // P)          # 49
        self.NS = self.NT * P                   # 6272
        self.NPAIR = (self.NT + 1) // 2         # 25
        self.KC = DF // P
        self.NR = [RT[r] * P for r in range(3)]             # rows/core/range
        self.NRF = [self.NR[r] * M for r in range(3)]       # table rows
        assert max(self.NRF) <= 32767
        self.X2W = 2 * DID
        # filled by prep:
        self.perm = None          # [M] arrays: orig local -> padded pos
        self.iperm = None
        # per (pair, range): chunk count, idx offsets, dstl offsets, mm plans
        self.ct = None            # [NPAIR][3]
        self.idx_off8 = None      # [NPAIR][3] col offset (units of 8) in idx sbuf
        self.dstl_off = None      # [NPAIR][3] col offset in dstl sbuf
        self.nbch = None          # [NPAIR][3] chunks that need B-half ohs
        self.combos = None        # [NPAIR][3][chunk] -> list of (half, th)
        self.maxct = None
        self.tot_ct = None
        self.tot_idx8 = None


# ----------------------------------------------------------------------------
# host-side input preparation
# ----------------------------------------------------------------------------

RAW_RB = None  # raw-local-id range boundaries, filled in _prep_edges


def _balance_perm(cfg, deg):
    """deg: [NS_RAW, 3] per-node per-src-range incoming edge counts for one
    core.  Source ranges are FIXED by raw local id, so only the dst tile
    assignment is balanced: per range-block, greedy vector bin-pack into its
    tiles.  Returns orig local id -> padded position (tile*128 + slot)."""
    pos = np.empty(cfg.NS_RAW, np.int64)
    for r in range(3):
        lo = RAW_RB[r]
        hi = min(RAW_RB[r + 1], cfg.NS_RAW)
        nodes = np.arange(lo, hi)
        ntile = RT[r]
        d = deg[nodes].astype(np.float64)
        order = np.argsort(-d.sum(axis=1), kind="stable")
        load = np.zeros((ntile, 3), np.float64)
        cnt = np.zeros(ntile, np.int32)
        tgt = d.sum(axis=0) / ntile + 1e-9
        for i in order:
            dv = d[i]
            score = (((load + dv) / tgt) ** 2).sum(axis=1)
            score[cnt >= P] = np.inf
            t = int(np.argmin(score))
            load[t] += dv
            pos[nodes[i]] = (RB[r] + t) * P + cnt[t]
            cnt[t] += 1
    return pos


def _prep_edges(cfg, edge_index):
    global RAW_RB
    M, NT, NP_ = cfg.M, cfg.NT, cfg.NPAIR
    RAW_RB = [0, RT[0] * P, (RT[0] + RT[1]) * P, cfg.NS_RAW]
    src = np.asarray(edge_index[0], dtype=np.int64)
    dst = np.asarray(edge_index[1], dtype=np.int64)
    core_d = dst // cfg.NS_RAW
    loc_d = dst % cfg.NS_RAW
    core_s = src // cfg.NS_RAW
    loc_s = src % cfg.NS_RAW

    # source range fixed by RAW local id (so balancing has no circularity)
    rng = np.digitize(loc_s, RAW_RB[1:3])        # 0/1/2
    deg_all = np.zeros((M, cfg.NS_RAW, 3), np.int64)
    np.add.at(deg_all, (core_d, loc_d, rng), 1)
    perm = [_balance_perm(cfg, deg_all[c]) for c in range(M)]
    cfg.perm = perm
    cfg.iperm = [np.argsort(p, kind="stable") for p in perm]

    # positions after permutation
    allperm = np.concatenate([perm[c] for c in range(M)])
    pos_s = allperm[src]
    pos_d = allperm[dst]
    srow = np.empty(cfg.E, np.int64)              # table row within range
    for r in range(3):
        m = rng == r
        srow[m] = core_s[m] * cfg.NR[r] + (pos_s[m] - RB[r] * P)
    tile_d = pos_d // P
    pair_d = tile_d // 2
    dslot = (pos_d % P) + (tile_d % 2) * P        # 0..255 within pair

    # group edges by (core_d, pair, range)
    gid = (core_d * NP_ + pair_d) * 3 + rng
    order = np.argsort(gid, kind="stable")
    ngroups = M * NP_ * 3
    counts = np.bincount(gid, minlength=ngroups)
    starts = np.zeros(ngroups + 1, np.int64)
    np.cumsum(counts, out=starts[1:])
    srow_s = srow[order]
    dslot_s = dslot[order]

    # build descriptor lists per group: (idx, slotA, slotB)
    desc_g = {}
    for c in range(M):
        for k in range(NP_):
            for r in range(3):
                g = (c * NP_ + k) * 3 + r
                s0, s1 = starts[g], starts[g + 1]
                rows = srow_s[s0:s1]
                slots = dslot_s[s0:s1]
                o = np.argsort(rows, kind="stable")
                rows = rows[o]; slots = slots[o]
                descs = []
                n = len(rows)
                used = np.zeros(n, bool)
                i = 0
                # pointers for pairing: for each i, try partner with row+1
                j = 0
                for i in range(n):
                    if used[i]:
                        continue
                    ri = rows[i]
                    # find an unused edge with row ri+1 (rows sorted)
                    j = max(j, i + 1)
                    while j < n and (rows[j] < ri + 1 or used[j]):
                        j += 1
                    if j < n and rows[j] == ri + 1 and not used[j]:
                        used[i] = used[j] = True
                        descs.append((ri, slots[i], slots[j]))
                    elif ri == cfg.NRF[r] - 1:
                        # last table row: put on B-half of desc at ri-1
                        used[i] = True
                        descs.append((ri - 1, -1, slots[i]))
                    else:
                        used[i] = True
                        descs.append((ri, slots[i], -1))
                # sort: B-carrying first, then by A-tile half
                descs.sort(key=lambda d: (0 if d[2] >= 0 else 1,
                                          (d[1] if d[1] >= 0 else d[2]) // P))
                desc_g[(c, k, r)] = descs

    cfg._desc_g = desc_g  # exposed for host-side validation

    # per (pair, range): chunk count = ceil(max-core ndesc / 128)
    ct = [[0] * 3 for _ in range(NP_)]
    nbch = [[0] * 3 for _ in range(NP_)]
    for k in range(NP_):
        for r in range(3):
            mx = max(len(desc_g[(c, k, r)]) for c in range(M))
            ct[k][r] = max(1, -(-mx // P))
            nb = max(sum(1 for d in desc_g[(c, k, r)] if d[2] >= 0)
                     for c in range(M))
            nbch[k][r] = -(-nb // P) if nb else 0
    cfg.ct = ct
    cfg.nbch = nbch
    cfg.maxct = max(max(row) for row in ct)
    cfg.maxnb = max(1, max(max(row) for row in nbch))
    idx_off = [[0] * 3 for _ in range(NP_)]
    dstl_off = [[0] * 3 for _ in range(NP_)]
    tot = 0
    # idx/dstl laid out in EMISSION order: sweep r, then pair
    for r in range(3):
        for k in range(NP_):
            idx_off[k][r] = tot
            dstl_off[k][r] = tot
            tot += ct[k][r]
    cfg.tot_ct = tot
    cfg.idx_off8 = [[idx_off[k][r] * 8 for r in range(3)] for k in range(NP_)]
    cfg.dstl_off = dstl_off
    cfg.tot_idx8 = tot * 8

    # combos per chunk: which (half, tile_half) matmuls to emit
    combos = [[None] * 3 for _ in range(NP_)]
    aspan = [[None] * 3 for _ in range(NP_)]
    for k in range(NP_):
        for r in range(3):
            nch = ct[k][r]
            pres = np.zeros((nch, 2, 2), bool)   # [chunk, half(A=0,B=1), th]
            for c in range(M):
                for i, (ri, sa, sb) in enumerate(desc_g[(c, k, r)]):
                    ch = i // P
                    if sa >= 0:
                        pres[ch, 0, sa // P] = True
                    if sb >= 0:
                        pres[ch, 1, sb // P] = True
            combos[k][r] = [
                [(h, th) for h in range(2) for th in range(2) if pres[ch, h, th]]
                for ch in range(nch)
            ]
            a0 = [ch for ch in range(nch) if pres[ch, 0, 0]]
            a1 = [ch for ch in range(nch) if pres[ch, 0, 1]]
            aspan[k][r] = (max(a0) + 1 if a0 else 0,
                           min(a1) if a1 else nch)
    cfg.combos = combos
    cfg.aspan = aspan

    # pack idx + dstl tensors per core
    idx_sb, dstlA_sb, dstlB_sb = [], [], []
    for c in range(M):
        flat = np.zeros(tot * P, np.int16)
        dA = np.full((tot, P), -1.0, np.float32)
        dB = np.full((tot, P), -1.0, np.float32)
        for k in range(NP_):
            for r in range(3):
                descs = desc_g[(c, k, r)]
                o = dstl_off[k][r]
                for i, (ri, sa, sb) in enumerate(descs):
                    flat[o * P + i] = ri
                    dA[o + i // P, i % P] = sa
                    dB[o + i // P, i % P] = sb
        blk = flat.reshape(-1, 16).T.copy()
        idx_sb.append(np.tile(blk, (8, 1)))
        dstlA_sb.append(dA.T.copy())
        dstlB_sb.append(dB.T.copy())
    return idx_sb, dstlA_sb, dstlB_sb


def _tile_rows(a, cfg):
    D = a.shape[1]
    return (a.reshape(cfg.NT, P, D).transpose(1, 0, 2).reshape(P, cfg.NT * D)
            .copy())


def _bf16(a):
    import ml_dtypes
    return np.asarray(a, np.float32).astype(ml_dtypes.bfloat16)


def prep_inputs(cfg, inputs):
    f32 = np.float32
    feats = np.asarray(inputs["features"], f32)
    id_emb = np.asarray(inputs["id_embedding"], f32)
    W_mlp = np.asarray(inputs["W_mlp"], f32)
    b_mlp = np.asarray(inputs["b_mlp"], f32)
    W_c1 = np.asarray(inputs["W_conv1"], f32)
    W_l1 = np.asarray(inputs["W_lin1"], f32)
    b_l1 = np.asarray(inputs["b_lin1"], f32)
    W_g1 = np.asarray(inputs["W_g1"], f32)
    b_g1 = np.asarray(inputs["b_g1"], f32)
    W_c2 = np.asarray(inputs["W_conv2"], f32)
    W_l2 = np.asarray(inputs["W_lin2"], f32)
    b_l2 = np.asarray(inputs["b_lin2"], f32)
    W_g2 = np.asarray(inputs["W_g2"], f32)
    b_g2 = np.asarray(inputs["b_g2"], f32)

    idx_sb, dstlA_sb, dstlB_sb = _prep_edges(cfg, inputs["edge_index"])

    wmlpT = W_mlp.T
    wfold = wmlpT @ W_l1.T
    wext = np.concatenate([wmlpT, wfold], axis=1)
    bext = np.concatenate([b_mlp, b_mlp @ W_l1.T], axis=0)

    iota = np.broadcast_to(np.arange(P, dtype=f32), (P, P)).copy()
    iotaw = np.broadcast_to(np.tile(np.arange(P, dtype=f32), cfg.maxct),
                            (P, cfg.maxct * P)).copy()
    shared = {
        "iotaw": _bf16(iotaw),
        "iotaw2": _bf16(iotaw + 128.0),
        "wext": _bf16(wext),
        "wc1T": _bf16(W_c1.T),
        "wg1T": _bf16(W_g1.T),
        "wc2T": _bf16(W_c2.T),
        "wlin2T": _bf16(W_l2.T),
        "wg2T": _bf16(W_g2.T),
        "bext": np.broadcast_to(bext, (P, cfg.DLX)).copy(),
        "blin1": np.broadcast_to(b_l1, (P, cfg.DID)).copy(),
        "blin2": np.broadcast_to(b_l2, (P, cfg.DID)).copy(),
        "iota": _bf16(iota),
    }

    in_maps = []
    for c in range(cfg.M):
        r0, r1 = c * cfg.NS_RAW, (c + 1) * cfg.NS_RAW
        featT = np.zeros((cfg.DF, cfg.NS), f32)
        featT[:, cfg.perm[c]] = feats[r0:r1].T
        idp = np.zeros((cfg.NS, cfg.DID), f32)
        idp[cfg.perm[c]] = id_emb[r0:r1]
        m = dict(shared)
        m["featT"] = _bf16(featT)
        m["id1"] = _tile_rows(idp + b_g1, cfg)
        m["id2"] = _tile_rows(idp + b_g2, cfg)
        m["gidx"] = idx_sb[c]
        m["dstlA"] = _bf16(dstlA_sb[c])
        m["dstlB"] = _bf16(dstlB_sb[c])
        in_maps.append(m)
    return in_maps


# ----------------------------------------------------------------------------
# bass kernel
# ----------------------------------------------------------------------------

def build_bass(cfg, skip_bias_mlp, skip_bias_lin2):
    import concourse.mybir as mybir
    import concourse.tile as tile
    from concourse.masks import make_identity

    f32 = mybir.dt.float32
    bf16 = mybir.dt.bfloat16
    i16 = mybir.dt.int16
    DL, DID, DF, DLX = cfg.DL, cfg.DID, cfg.DF, cfg.DLX
    NT, KC, NP_ = cfg.NT, cfg.KC, cfg.NPAIR
    X2W = cfg.X2W
    Act = mybir.ActivationFunctionType
    Op = mybir.AluOpType

    nc = _make_nc(cfg.M)
    featT = nc.dram_tensor("featT", [DF, cfg.NS], bf16, kind="ExternalInput")
    wext = nc.dram_tensor("wext", [DF, DLX], bf16, kind="ExternalInput")
    wc1T = nc.dram_tensor("wc1T", [DL, DL], bf16, kind="ExternalInput")
    wg1T = nc.dram_tensor("wg1T", [DL, DID], bf16, kind="ExternalInput")
    wc2T = nc.dram_tensor("wc2T", [DID, DID], bf16, kind="ExternalInput")
    wlin2T = nc.dram_tensor("wlin2T", [DID, DID], bf16, kind="ExternalInput")
    wg2T = nc.dram_tensor("wg2T", [DID, DID], bf16, kind="ExternalInput")
    bext = nc.dram_tensor("bext", [P, DLX], f32, kind="ExternalInput")
    blin1 = nc.dram_tensor("blin1", [P, DID], f32, kind="ExternalInput")
    blin2 = nc.dram_tensor("blin2", [P, DID], f32, kind="ExternalInput")
    iota = nc.dram_tensor("iota", [P, P], bf16, kind="ExternalInput")
    id1 = nc.dram_tensor("id1", [P, NT * DID], f32, kind="ExternalInput")
    id2 = nc.dram_tensor("id2", [P, NT * DID], f32, kind="ExternalInput")
    gidx = nc.dram_tensor("gidx", [P, cfg.tot_idx8], i16, kind="ExternalInput")
    dstlA = nc.dram_tensor("dstlA", [P, cfg.tot_ct], bf16, kind="ExternalInput")
    dstlB = nc.dram_tensor("dstlB", [P, cfg.tot_ct], bf16, kind="ExternalInput")
    iotaw = nc.dram_tensor("iotaw", [P, cfg.maxct * P], bf16, kind="ExternalInput")
    iotaw2 = nc.dram_tensor("iotaw2", [P, cfg.maxct * P], bf16, kind="ExternalInput")
    out = nc.dram_tensor("out", [cfg.NS, DID], f32, kind="ExternalOutput")

    groups = [list(range(cfg.M))]

    with tile.TileContext(nc) as tc:
        with (
            tc.tile_pool(name="res", bufs=1) as res,
            tc.tile_pool(name="dram", bufs=1, space="DRAM") as dram,
        ):
            # ---------------- residents ----------------
            wc1_sb = res.tile([P, 2, DL], bf16)
            nc.sync.dma_start(out=wc1_sb[:], in_=wc1T.rearrange("(c p) n -> p c n", p=P))
            wg1_sb = res.tile([P, 2, DID], bf16)
            nc.sync.dma_start(out=wg1_sb[:], in_=wg1T.rearrange("(c p) n -> p c n", p=P))
            wc2_sb = res.tile([DID, DID], bf16)
            nc.sync.dma_start(out=wc2_sb[:], in_=wc2T[:, :])
            wlin2_sb = res.tile([DID, DID], bf16)
            nc.sync.dma_start(out=wlin2_sb[:], in_=wlin2T[:, :])
            wg2_sb = res.tile([DID, DID], bf16)
            nc.sync.dma_start(out=wg2_sb[:], in_=wg2T[:, :])
            bext_sb = res.tile([P, DLX], f32)
            nc.sync.dma_start(out=bext_sb[:], in_=bext[:, :])
            blin1_sb = res.tile([P, DID], f32)
            nc.sync.dma_start(out=blin1_sb[:], in_=blin1[:, :])
            blin2_sb = res.tile([P, DID], f32)
            nc.sync.dma_start(out=blin2_sb[:], in_=blin2[:, :])
            gidx_sb = res.tile([P, cfg.tot_idx8], i16)
            nc.sync.dma_start(out=gidx_sb[:], in_=gidx[:, :])
            dstlA_sb = res.tile([P, cfg.tot_ct], bf16)
            nc.sync.dma_start(out=dstlA_sb[:], in_=dstlA[:, :])
            dstlB_sb = res.tile([P, cfg.tot_ct], bf16)
            nc.sync.dma_start(out=dstlB_sb[:], in_=dstlB[:, :])
            iotaw_sb = res.tile([P, cfg.maxct, P], bf16)
            nc.sync.dma_start(out=iotaw_sb[:], in_=iotaw.rearrange("p (c d) -> p c d", d=P))
            iotaw2_sb = res.tile([P, cfg.maxct, P], bf16)
            nc.sync.dma_start(out=iotaw2_sb[:], in_=iotaw2.rearrange("p (c d) -> p c d", d=P))
            ident_sb = res.tile([P, P], bf16)
            make_identity(nc, ident_sb[:])
            xhat_sb = res.tile([P, NT * DID], bf16)
            s1acc = res.tile([P, NT, DL], f32)      # conv1 accumulator

            x_ag_in = [dram.tile([cfg.NR[r], DL], bf16, name=f"x_ag_in{r}")
                       for r in range(3)]
            x_full = [dram.tile([cfg.NRF[r], DL], bf16, addr_space="Shared",
                                name=f"x_full{r}") for r in range(3)]
            x2_ag_in = [dram.tile([cfg.NR[r], X2W], bf16, name=f"x2_ag_in{r}")
                        for r in range(3)]
            x2_full = [dram.tile([cfg.NRF[r], X2W], bf16, addr_space="Shared",
                                 name=f"x2_full{r}") for r in range(3)]

            APc = None

            def ov_ap(t, nrows, elem):
                nonlocal APc
                base = t[:, :]
                if APc is None:
                    APc = type(base)
                return APc(tensor=base.tensor, offset=base.offset,
                           ap=[[elem, nrows - 1], [1, 2 * elem]])

            _glp_cm = tc.tile_pool(name="conv_g", bufs=3)
            glp = _glp_cm.__enter__()
            _ohp_cm = tc.tile_pool(name="conv_oh", bufs=2)
            ohp = _ohp_cm.__enter__()
            _csb_cm = tc.tile_pool(name="conv_sb", bufs=2)
            csb = _csb_cm.__enter__()
            _aps1_cm = tc.tile_pool(name="aps1", bufs=2, space="PSUM")
            aps1 = _aps1_cm.__enter__()

            # ---------------- phase B: MLP + l2norm + x_hat ----------------
            featT_r = featT.rearrange("(c p) n -> p c n", p=P)
            with (
                tc.tile_pool(name="pbres", bufs=1) as pbres,
                tc.tile_pool(name="mlp_sb", bufs=2) as sb,
                tc.tile_pool(name="mlp_ps", bufs=4, space="PSUM") as ps,
            ):
                wext_sb = pbres.tile([P, KC, DLX], bf16)
                nc.sync.dma_start(out=wext_sb[:],
                                  in_=wext.rearrange("(c p) n -> p c n", p=P))
                SLAB = P
                n_slabs = -(-cfg.NS // SLAB)
                for s in range(n_slabs):
                    n0 = s * SLAB
                    W = min(SLAB, cfg.NS - n0)
                    feat_sb = sb.tile([P, KC, SLAB], bf16, tag="feat")
                    nc.sync.dma_start(out=feat_sb[:, :, :W],
                                      in_=featT_r[:, :, n0:n0 + W])
                    for h in range(W // P):
                        t = (n0 + h * P) // P
                        z = ps.tile([P, DLX], f32, tag="z")
                        for c in range(KC):
                            nc.tensor.matmul(
                                out=z[:],
                                lhsT=feat_sb[:, c, h * P:(h + 1) * P],
                                rhs=wext_sb[:, c, :],
                                start=(c == 0), stop=(c == KC - 1),
                            )
                        if skip_bias_mlp:
                            zb = z
                        else:
                            zb = sb.tile([P, DLX], f32, tag="zb")
                            nc.vector.tensor_add(out=zb[:], in0=z[:], in1=bext_sb[:])
                        sq = sb.tile([P, DL], f32, tag="sq")
                        ss = sb.tile([P, 1], f32, tag="ss")
                        nc.scalar.activation(out=sq[:], in_=zb[:, :DL],
                                             func=Act.Square, accum_out=ss[:])
                        ssc = sb.tile([P, 1], f32, tag="ssc")
                        nc.vector.tensor_scalar_max(out=ssc[:], in0=ss[:],
                                                    scalar1=1e-24)
                        sr = sb.tile([P, 1], f32, tag="sr")
                        nc.scalar.activation(out=sr[:], in_=ssc[:], func=Act.Sqrt)
                        rs = sb.tile([P, 1], f32, tag="rs")
                        nc.vector.reciprocal(out=rs[:], in_=sr[:])
                        xt = sb.tile([P, DL], bf16, tag="xt")
                        nc.vector.tensor_scalar_mul(out=xt[:], in0=zb[:, :DL],
                                                    scalar1=rs[:, :1])
                        rr = 0 if t < RB[1] else (1 if t < RB[2] else 2)
                        tb = t - RB[rr]
                        nc.sync.dma_start(
                            out=x_ag_in[rr][tb * P:(tb + 1) * P, :], in_=xt[:])
                        if t == RB[1] - 1:
                            nc.gpsimd.collective_compute(
                                "AllGather", Op.bypass, replica_groups=groups,
                                ins=[x_ag_in[0].opt()], outs=[x_full[0].opt()])
                        # x_hat
                        xl = sb.tile([P, DID], f32, tag="xl")
                        nc.vector.tensor_scalar_mul(out=xl[:], in0=zb[:, DL:DLX],
                                                    scalar1=rs[:, :1])
                        if skip_bias_lin2:
                            xlb = xl
                        else:
                            xlb = sb.tile([P, DID], f32, tag="xlb")
                            nc.vector.tensor_add(out=xlb[:], in0=xl[:], in1=blin1_sb[:])
                        xh1 = sb.tile([P, DID], f32, tag="xh1")
                        nc.vector.scalar_tensor_tensor(
                            out=xh1[:], in0=xlb[:], scalar=0.01, in1=xlb[:],
                            op0=Op.mult, op1=Op.max)
                        id1t = sb.tile([P, DID], f32, tag="id1t")
                        nc.sync.dma_start(out=id1t[:],
                                          in_=id1[:, t * DID:(t + 1) * DID])
                        nc.vector.tensor_add(out=xhat_sb[:, t * DID:(t + 1) * DID],
                                             in0=xh1[:], in1=id1t[:])
            _res2_cm = tc.tile_pool(name="res2", bufs=1)
            res2 = _res2_cm.__enter__()
            x2T_sb = res2.tile([DID, NT * P], bf16)
            s2acc = res2.tile([DID, NT, P], bf16)   # conv2 accumulator (s2T)


            # ---------------- conv sweeps (shared emitter) ----------------
            def conv_sweeps(layer, x_tabs, elem, acc, acc_w, finish):
                """layer 1: elem=DL, acc=s1acc [P, NT, DL]
                   layer 2: elem=X2W, acc=s2acc [DID, NT, P] (transposed)"""
                started = [[False, False] for _ in range(NP_)]
                with (
                    tc.tile_pool(name=f"c{layer}_ps", bufs=2, space="PSUM") as ps_in,
                    tc.tile_pool(name=f"c{layer}_psf", bufs=1, space="PSUM") as psf,
                    tc.tile_pool(name=f"c{layer}_g2", bufs=3) as glp2,
                ):
                    ps = aps1 if layer == 1 else ps_in
                    gp_use = glp if layer == 1 else glp2
                    LEAD = 2
                    def emit_gather(k, r):
                        ctk = cfg.ct[k][r]
                        gl = gp_use.tile([P, cfg.maxct, 2 * elem], bf16,
                                         tag=f"gl{layer}")
                        o8 = cfg.idx_off8[k][r]
                        nc.gpsimd.dma_gather(
                            gl[:, :ctk, :], ov_ap(x_tabs[r], cfg.NRF[r], elem),
                            gidx_sb[:, o8:o8 + ctk * 8],
                            ctk * P, ctk * P, 2 * elem, elem_step=elem,
                            single_packet=False, queue_num=0,
                        )
                        return gl

                    sched = [(k, r) for r in range(3) for k in range(NP_)]
                    q = []
                    for i in range(min(LEAD, len(sched))):
                        q.append(emit_gather(*sched[i]))
                    for i, (k, r) in enumerate(sched):
                        if layer == 1 and i in (12, 20):
                            r_ = 1 if i == 12 else 2
                            nc.gpsimd.collective_compute(
                                "AllGather", Op.bypass, replica_groups=groups,
                                ins=[x_ag_in[r_].opt()],
                                outs=[x_full[r_].opt()])
                        gl = q.pop(0)
                        if i + LEAD < len(sched):
                            q.append(emit_gather(*sched[i + LEAD]))
                        ctk = cfg.ct[k][r]
                        dof = cfg.dstl_off[k][r]
                        nbc = cfg.nbch[k][r]
                        # batched one-hots: A halves for all chunks, both tile
                        # halves; B halves for the B-prefix chunks.
                        cA0, sA1 = cfg.aspan[k][r]
                        ohA0 = ohp.tile([P, cfg.maxct, P], bf16, tag="ohA0")
                        if cA0 > 0:
                            nc.vector.tensor_tensor(
                                out=ohA0[:, :cA0, :],
                                in0=dstlA_sb[:, dof:dof + cA0].to_broadcast([P, cA0, P]),
                                in1=iotaw_sb[:, :cA0, :], op=Op.is_equal)
                        ohA1 = ohp.tile([P, cfg.maxct, P], bf16, tag="ohA1")
                        if sA1 < ctk:
                            nA1 = ctk - sA1
                            nc.vector.tensor_tensor(
                                out=ohA1[:, sA1:ctk, :],
                                in0=dstlA_sb[:, dof + sA1:dof + ctk].to_broadcast([P, nA1, P]),
                                in1=iotaw2_sb[:, :nA1, :], op=Op.is_equal)
                        if nbc:
                            nbk = cfg.maxnb
                            ohB0 = ohp.tile([P, nbk, P], bf16, tag="ohB0")
                            nc.vector.tensor_tensor(
                                out=ohB0[:, :nbc, :],
                                in0=dstlB_sb[:, dof:dof + nbc].to_broadcast([P, nbc, P]),
                                in1=iotaw_sb[:, :nbc, :], op=Op.is_equal)
                            ohB1 = ohp.tile([P, nbk, P], bf16, tag="ohB1")
                            nc.vector.tensor_tensor(
                                out=ohB1[:, :nbc, :],
                                in0=dstlB_sb[:, dof:dof + nbc].to_broadcast([P, nbc, P]),
                                in1=iotaw2_sb[:, :nbc, :], op=Op.is_equal)
                        oh_map = {(0, 0): ohA0, (0, 1): ohA1}
                        if nbc:
                            oh_map[(1, 0)] = ohB0
                            oh_map[(1, 1)] = ohB1
                        # per tile-half psum accumulate over this range sweep
                        pst = [None, None]
                        mm = [[], []]
                        for ch in range(ctk):
                            for (h, th) in cfg.combos[k][r][ch]:
                                mm[th].append((ch, h))
                        for th in (0, 1):
                            t = 2 * k + th
                            if t >= NT or not mm[th]:
                                continue
                            if layer == 1:
                                pt = ps.tile([P, DL], f32, tag=f"p{th}")
                            else:
                                pt = ps.tile([DID, P], f32, tag=f"p{th}")
                            pst[th] = pt
                            for idx_mm, (ch, h) in enumerate(mm[th]):
                                oh = oh_map[(h, th)]
                                rhs_sl = gl[:, ch, h * elem:h * elem + acc_w]
                                if layer == 1:
                                    nc.tensor.matmul(
                                        out=pt[:], lhsT=oh[:, ch, :], rhs=rhs_sl,
                                        start=(idx_mm == 0),
                                        stop=(idx_mm == len(mm[th]) - 1))
                                else:
                                    nc.tensor.matmul(
                                        out=pt[:], lhsT=rhs_sl, rhs=oh[:, ch, :],
                                        start=(idx_mm == 0),
                                        stop=(idx_mm == len(mm[th]) - 1))
                            # accumulate into SBUF
                            if layer == 1:
                                a_sl = acc[:, t, :]
                            else:
                                a_sl = acc[:, t, :]
                            if not started[k][th]:
                                nc.vector.tensor_copy(out=a_sl, in_=pt[:])
                                started[k][th] = True
                            else:
                                nc.vector.tensor_add(out=a_sl, in0=a_sl, in1=pt[:])
                        if r == 2:
                            for th in (0, 1):
                                t = 2 * k + th
                                if t >= NT:
                                    continue
                                finish(t, csb, psf)

            # ---------------- layer 1 ----------------
            def finish1(t, sb, ps):
                s1_sb = sb.tile([P, DL], bf16, tag="s1sb")
                nc.vector.tensor_copy(out=s1_sb[:], in_=s1acc[:, t, :])
                s1T = sb.tile([P, 2, P], bf16, tag="s1T")
                for fh in range(2):
                    tp = ps.tile([P, P], bf16, tag="t128")
                    nc.tensor.transpose(out=tp[:], in_=s1_sb[:, fh * P:(fh + 1) * P],
                                        identity=ident_sb[:])
                    nc.vector.tensor_copy(out=s1T[:, fh, :], in_=tp[:])
                h1T = sb.tile([P, 2, P], bf16, tag="h1T")
                for oc in range(2):
                    h1p = ps.tile([P, P], f32, tag="h1p")
                    for fh in range(2):
                        nc.tensor.matmul(
                            out=h1p[:], lhsT=wc1_sb[:, fh, oc * P:(oc + 1) * P],
                            rhs=s1T[:, fh, :],
                            start=(fh == 0), stop=(fh == 1))
                    nc.scalar.activation(out=h1T[:, oc, :], in_=h1p[:],
                                         func=Act.Lrelu, alpha=0.01)
                x2p = ps.tile([P, DID], f32, tag="x2p")
                for oc in range(2):
                    nc.tensor.matmul(out=x2p[:], lhsT=h1T[:, oc, :],
                                     rhs=wg1_sb[:, oc, :],
                                     start=(oc == 0), stop=(oc == 1))
                x2a = sb.tile([P, DID], f32, tag="x2a")
                nc.vector.tensor_add(out=x2a[:], in0=x2p[:],
                                     in1=xhat_sb[:, t * DID:(t + 1) * DID])
                x2pad = sb.tile([P, X2W], bf16, tag="x2pad")
                nc.scalar.activation(out=x2pad[:, :DID], in_=x2a[:],
                                     func=Act.Lrelu, alpha=0.01)
                rr = 0 if t < RB[1] else (1 if t < RB[2] else 2)
                tb = t - RB[rr]
                nc.sync.dma_start(out=x2_ag_in[rr][tb * P:(tb + 1) * P, :],
                                  in_=x2pad[:])
                if t == RB[1] + 2 or t == RB[2] + 2 or t == NT - 1:
                    r_ = 0 if t == RB[1] + 2 else (1 if t == RB[2] + 2 else 2)
                    nc.gpsimd.collective_compute(
                        "AllGather", Op.bypass, replica_groups=groups,
                        ins=[x2_ag_in[r_].opt()], outs=[x2_full[r_].opt()])
                x2Tp = ps.tile([DID, P], bf16, tag="x2Tp")
                nc.tensor.transpose(out=x2Tp[:], in_=x2pad[:, :DID],
                                    identity=ident_sb[:])
                nc.vector.tensor_copy(out=x2T_sb[:, t * P:(t + 1) * P],
                                      in_=x2Tp[:])

            conv_sweeps(1, x_full, DL, s1acc, DL, finish1)
            _aps1_cm.__exit__(None, None, None)

            # ---------------- layer 2 ----------------
            def finish2(t, sb, ps):
                s2T_sb = sb.tile([DID, P], bf16, tag="s2sb")
                nc.vector.tensor_copy(out=s2T_sb[:], in_=s2acc[:, t, :])
                h2p = ps.tile([DID, P], f32, tag="pa2")
                nc.tensor.matmul(out=h2p[:], lhsT=wc2_sb[:], rhs=s2T_sb[:],
                                 start=True, stop=True)
                h2T = sb.tile([DID, P], bf16, tag="h2T")
                nc.scalar.activation(out=h2T[:], in_=h2p[:],
                                     func=Act.Lrelu, alpha=0.01)
                xh2 = ps.tile([P, DID], f32, tag="pb")
                nc.tensor.matmul(out=xh2[:], lhsT=x2T_sb[:, t * P:(t + 1) * P],
                                 rhs=wlin2_sb[:], start=True, stop=True)
                xh2b = sb.tile([P, DID], f32, tag="xh2b")
                if skip_bias_lin2:
                    nc.scalar.activation(out=xh2b[:], in_=xh2[:],
                                         func=Act.Lrelu, alpha=0.01)
                else:
                    xh2a = sb.tile([P, DID], f32, tag="xh2a")
                    nc.vector.tensor_add(out=xh2a[:], in0=xh2[:], in1=blin2_sb[:])
                    nc.scalar.activation(out=xh2b[:], in_=xh2a[:],
                                         func=Act.Lrelu, alpha=0.01)
                id2t = sb.tile([P, DID], f32, tag="id2t")
                nc.sync.dma_start(out=id2t[:],
                                  in_=id2[:, t * DID:(t + 1) * DID])
                xhat2 = sb.tile([P, DID], f32, tag="xhat2")
                nc.vector.tensor_add(out=xhat2[:], in0=xh2b[:], in1=id2t[:])
                op_ = ps.tile([P, DID], f32, tag="pb2")
                nc.tensor.matmul(out=op_[:], lhsT=h2T[:], rhs=wg2_sb[:],
                                 start=True, stop=True)
                o1 = sb.tile([P, DID], f32, tag="o1")
                nc.vector.tensor_add(out=o1[:], in0=op_[:], in1=xhat2[:])
                o2 = sb.tile([P, DID], f32, tag="o2")
                nc.scalar.activation(out=o2[:], in_=o1[:],
                                     func=Act.Lrelu, alpha=0.01)
                nc.sync.dma_start(out=out[t * P:(t + 1) * P, :], in_=o2[:])

            conv_sweeps(2, x2_full, X2W, s2acc, DID, finish2)

            _res2_cm.__exit__(None, None, None)
            _csb_cm.__exit__(None, None, None)
            _ohp_cm.__exit__(None, None, None)
            _glp_cm.__exit__(None, None, None)

    return nc


# ----------------------------------------------------------------------------
# entry points
# ----------------------------------------------------------------------------

LAST_EXEC_NS = None


def run(cfg, inputs, trace=False):
    global LAST_EXEC_NS
    _install_ntff_shim()
    from concourse.bass_utils import run_bass_kernel_spmd

    in_maps = prep_inputs(cfg, inputs)
    skip_bias_mlp = not np.any(np.asarray(inputs["b_mlp"]))
    skip_bias_lin2 = (not np.any(np.asarray(inputs["b_lin1"]))
                      and not np.any(np.asarray(inputs["b_lin2"])))
    nc = build_bass(cfg, skip_bias_mlp, skip_bias_lin2)
    nc.finalize()
    res = run_bass_kernel_spmd(nc, in_maps, list(range(cfg.M)), trace=trace)
    LAST_EXEC_NS = res.exec_time_ns
    outs = []
    for c in range(cfg.M):
        o = res.results[c]["out"]          # [NS, DID] in permuted order
        outs.append(o[cfg.perm[c]])        # back to original local order
    return np.concatenate(outs, axis=0)


def kernel(**inputs):
    trace = bool(os.environ.get("GCN_TRACE"))
    return run(Cfg(), inputs, trace=trace)
